# revision 41
# baseline (speedup 1.0000x reference)
"""Trainium2 Bass kernel for nn_MemoryAttention (sparse_attention).

Reference computation (B=8, T=1024, C=512, H=8, D=64, T2=512):
    kv = x @ W_kv ; k, v = split(kv)
    sk = stack([roll(k[:, :T2], i, axis=0) for i in range(7)]).reshape(B, 7*T2, C)
    K = concat(sk, k, axis=1)  # [B, S=4608, C]   (same for V)
    y = softmax(q K^T / sqrt(D)) V  (per head, unmasked)
    out = y @ W_proj

Sharding: core b owns batch b end-to-end; no cross-core communication.

The roll/stack/reshape memory block m (of 7) for batch b is k/v-half of batch
src(b, m) = ((b*7+m) % 8 - (b*7+m)//8) % 8.  The 7 sources always contain a
duplicate (a repeated source and/or the own batch, whose half is also in the
concatenated full-k tail), so attention only needs 6 distinct memory slots +
the own full block, with per-block integer weights w: a weighted key block
contributes w*exp(s) = exp(s + ln w), folded in via the activation bias input
(a host-built per-core [128, 32] bias table; padding slots use bias -60 ->
exp ~ 0).

Layout strategy (zero on-chip transposes):
  - host supplies x^T and q^T (and the 6 memory-slot x-half transposes)
  - k^T [C,T] comes straight out of the projection (W_k as lhsT, x^T as rhs)
  - v [T,C] natural (x^T as lhsT, W_v as rhs), stored per-head with an extra
    ones column -> the PV matmul also produces the softmax row-sums
  - scores computed transposed S^T[s,l]; unmasked softmax needs no
    max-subtraction here (|scores|/8 <= ~3); exp(S^T) feeds PV directly as rhs
  - per-head y^T [64,T] slices stack into y^T [C,T] = the lhsT of the output
    projection; out [T,C] emerges in natural layout.
"""

import os
import sys

for _p in ("/opt/trn_rl_repo", "/root/.axon_site/_ro/trn_rl_repo"):
    if os.path.isdir(_p) and _p not in sys.path:
        sys.path.insert(0, _p)

import numpy as np
import ml_dtypes

B, T, C, H = 8, 1024, 512, 8
D = C // H          # 64
T2 = T // 2         # 512
NSLOT = 6           # distinct memory-source slots
NCORES = 8
CT = C // 128       # 4 contraction chunks
ST = NSLOT * 4 + T // 128   # 32 s-tiles
VW = D + 1          # 65 = v head width + ones column

BF16 = ml_dtypes.bfloat16
FP8 = ml_dtypes.float8_e4m3
# fp8 + DoubleRow for the memory-slot K/V projections: halves their PE time
# but CoreSim-measured error is 4.2e-2 (vs 4.4e-3 bf16) — too risky. Off.
FP8_SLOTS = False

_CACHE = {}
LAST_RESULTS = None  # test.py reads exec_time_ns from here


def _emit(nc, tc, mybir):
    from contextlib import ExitStack

    fp32 = mybir.dt.float32
    bf16 = mybir.dt.bfloat16
    Exp = mybir.ActivationFunctionType.Exp

    fp8 = mybir.dt.float8e4
    hdt = fp8 if FP8_SLOTS else bf16
    xT_d = nc.dram_tensor("xT", [C, T], bf16, kind="ExternalInput").ap()
    xhT_d = nc.dram_tensor("xhT", [NSLOT, C, T2], hdt, kind="ExternalInput").ap()
    if FP8_SLOTS:
        wk8_d = nc.dram_tensor("wk8", [C, C], fp8, kind="ExternalInput").ap()
        wv8_d = nc.dram_tensor("wv8", [C, C], fp8, kind="ExternalInput").ap()
    qT_d = nc.dram_tensor("qT", [C, T], bf16, kind="ExternalInput").ap()
    wk_d = nc.dram_tensor("wk", [C, C], bf16, kind="ExternalInput").ap()
    wv_d = nc.dram_tensor("wv", [C, C], bf16, kind="ExternalInput").ap()
    wp_d = nc.dram_tensor("wp", [C, C], bf16, kind="ExternalInput").ap()
    wb_d = nc.dram_tensor("wbias", [128, ST], fp32, kind="ExternalInput").ap()
    out_d = nc.dram_tensor("out", [T, C], fp32, kind="ExternalOutput").ap()

    with ExitStack() as ctx:
        persist = ctx.enter_context(tc.tile_pool(name="persist", bufs=1))
        attn_pool = ctx.enter_context(tc.tile_pool(name="attn", bufs=5))
        misc = ctx.enter_context(tc.tile_pool(name="misc", bufs=1))
        psA = ctx.enter_context(tc.tile_pool(name="psA", bufs=2, space="PSUM"))
        psP = ctx.enter_context(tc.tile_pool(name="psP", bufs=2, space="PSUM"))
        psY = ctx.enter_context(tc.tile_pool(name="psY", bufs=1, space="PSUM"))

        # ---------------- persistent SBUF ----------------
        xT = persist.tile([128, CT, T], bf16, tag="xT")
        qT = persist.tile([128, CT, T], bf16, tag="qT")
        xhT = persist.tile([128, CT, NSLOT, T2], hdt, tag="xhT")
        if FP8_SLOTS:
            wk8 = persist.tile([128, CT, C], fp8, tag="wk8")
            wv8 = persist.tile([128, CT, C], fp8, tag="wv8")
        wk = persist.tile([128, CT, C], bf16, tag="wk")
        wv = persist.tile([128, CT, C], bf16, tag="wv")
        wp = persist.tile([128, CT, C], bf16, tag="wp")
        wb = persist.tile([128, ST], fp32, tag="wb")
        kT = persist.tile([128, CT, T], bf16, tag="kT")
        kTh = persist.tile([128, NSLOT, CT, T2], bf16, tag="kTh")
        vown = persist.tile([128, T // 128, H, VW], bf16, tag="vown")
        vhalf = persist.tile([128, T2 // 128, NSLOT, H, VW], bf16, tag="vhalf")
        yT = persist.tile([128, CT, T], bf16, tag="yT")
        out_acc = persist.tile([128, T // 128, C], fp32, tag="out_acc")

        # ---------------- input DMAs (critical-path order) ----------------
        # kT-own proj needs wk+xT first; v-own needs wv; then slot 0, qT for
        # the first QK, remaining slots; wp only needed at the end.
        def _cs(ct):
            return slice(ct * 128, (ct + 1) * 128)

        for ct in range(CT):
            nc.sync.dma_start(wk[:, ct, :], wk_d[_cs(ct), :])
            nc.sync.dma_start(xT[:, ct, :], xT_d[_cs(ct), :])
        for ct in range(CT):
            nc.sync.dma_start(wv[:, ct, :], wv_d[_cs(ct), :])
        if FP8_SLOTS:
            for ct in range(CT):
                nc.sync.dma_start(wk8[:, ct, :], wk8_d[_cs(ct), :])
                nc.sync.dma_start(wv8[:, ct, :], wv8_d[_cs(ct), :])
        for ct in range(CT):
            nc.sync.dma_start(xhT[:, ct, 0, :], xhT_d[0, _cs(ct), :])
        nc.sync.dma_start(wb[:], wb_d[:, :])
        for ct in range(CT):
            nc.sync.dma_start(qT[:, ct, :], qT_d[_cs(ct), :])
        for j in range(1, NSLOT):
            for ct in range(CT):
                nc.sync.dma_start(xhT[:, ct, j, :], xhT_d[j, _cs(ct), :])
        for ct in range(CT):
            nc.sync.dma_start(wp[:, ct, :], wp_d[_cs(ct), :])

        # ones columns of the augmented V storage
        for tt in range(T // 128):
            nc.vector.memset(vown[:, tt, :, D], 1.0)
        for tt in range(T2 // 128):
            for j in range(NSLOT):
                nc.vector.memset(vhalf[:, tt, j, :, D], 1.0)

        # warm the ACT exp table during the initial DMA wait (walrus inserts
        # the ~2.7us ACT_TABLE_LOAD before the first ACTIVATE)
        warm = misc.tile([128, 8], fp32, tag="warm")
        nc.vector.memset(warm[0:1, 0:8], 0.0)
        nc.scalar.activation(warm[0:1, 0:8], warm[0:1, 0:8], Exp)

        # ---------------- projection helpers ----------------
        def proj_kT_own(jts):
            # kT[j, t] = sum_c wk[c, j] * xT[c, t]
            for jt in jts:
                for tch in range(2):
                    ps = psP.tile([128, 512], fp32, tag="psP")
                    for cc in range(CT):
                        nc.tensor.matmul(
                            ps[:],
                            wk[:, cc, jt * 128:(jt + 1) * 128],
                            xT[:, cc, tch * 512:(tch + 1) * 512],
                            start=(cc == 0),
                            stop=(cc == CT - 1),
                        )
                    nc.vector.tensor_copy(
                        kT[:, jt, tch * 512:(tch + 1) * 512], ps[:]
                    )

        def proj_v_own():
            # v[t, j] = sum_c xT[c, t] * wv[c, j], per-head into [., h, 0:64]
            for tt in range(T // 128):
                ps = psP.tile([128, 512], fp32, tag="psP")
                for cc in range(CT):
                    nc.tensor.matmul(
                        ps[:],
                        xT[:, cc, tt * 128:(tt + 1) * 128],
                        wv[:, cc, :],
                        start=(cc == 0),
                        stop=(cc == CT - 1),
                    )
                nc.vector.tensor_copy(
                    vown[:, tt, :, 0:D],
                    ps[:].rearrange("p (h d) -> p h d", h=H),
                )

        DR = mybir.MatmulPerfMode.DoubleRow

        def proj_slot(j):
            # fp8 DoubleRow: contract c in 256-wide pairs via 3D [Ki, 2, M]
            # APs — half the matmuls of the bf16 path.
            for jt in range(CT):
                ps = psP.tile([128, 512], fp32, tag="psP")
                if FP8_SLOTS:
                    for cp in range(CT // 2):
                        nc.tensor.matmul(
                            ps[:],
                            wk8[:, 2 * cp:2 * cp + 2, jt * 128:(jt + 1) * 128],
                            xhT[:, 2 * cp:2 * cp + 2, j, :],
                            start=(cp == 0),
                            stop=(cp == CT // 2 - 1),
                            perf_mode=DR,
                        )
                else:
                    for cc in range(CT):
                        nc.tensor.matmul(
                            ps[:],
                            wk[:, cc, jt * 128:(jt + 1) * 128],
                            xhT[:, cc, j, :],
                            start=(cc == 0),
                            stop=(cc == CT - 1),
                        )
                nc.vector.tensor_copy(kTh[:, j, jt, :], ps[:])
            for tt in range(T2 // 128):
                ps = psP.tile([128, 512], fp32, tag="psP")
                if FP8_SLOTS:
                    for cp in range(CT // 2):
                        nc.tensor.matmul(
                            ps[:],
                            xhT[:, 2 * cp:2 * cp + 2, j, tt * 128:(tt + 1) * 128],
                            wv8[:, 2 * cp:2 * cp + 2, :],
                            start=(cp == 0),
                            stop=(cp == CT // 2 - 1),
                            perf_mode=DR,
                        )
                else:
                    for cc in range(CT):
                        nc.tensor.matmul(
                            ps[:],
                            xhT[:, cc, j, tt * 128:(tt + 1) * 128],
                            wv[:, cc, :],
                            start=(cc == 0),
                            stop=(cc == CT - 1),
                        )
                nc.vector.tensor_copy(
                    vhalf[:, tt, j, :, 0:D],
                    ps[:].rearrange("p (h d) -> p h d", h=H),
                )

        # ---------------- attention ----------------
        # s-tile map: st < NSLOT*4 -> memory slot m=st//4, t-tile tt=st%4
        #             st >= NSLOT*4 -> own full k/v, t-tile tt=st-NSLOT*4
        def k_lhsT(h, st):
            p0 = (h % 2) * 64
            if st < NSLOT * 4:
                m, tt = st // 4, st % 4
                return kTh[p0:p0 + D, m, h // 2, tt * 128:(tt + 1) * 128]
            tt = st - NSLOT * 4
            return kT[p0:p0 + D, h // 2, tt * 128:(tt + 1) * 128]

        def v_lhsT(h, st):
            if st < NSLOT * 4:
                m, tt = st // 4, st % 4
                return vhalf[:, tt, m, h, :]
            tt = st - NSLOT * 4
            return vown[:, tt, h, :]

        scale = float(1.0 / np.sqrt(np.float32(D)))

        # s-tile processing order: own block first (its projections are tiny
        # and emitted first), then memory slots — lets head 0 start while the
        # slot projections stream in behind it.  Softmax/PV accumulation is
        # order-invariant; the bias table is indexed by the logical st.
        ORDER = list(range(NSLOT * 4, ST)) + list(range(NSLOT * 4))

        def attn_head(h, interleave=None, tail_cb=None):
            p0 = (h % 2) * 64
            y_ps = psY.tile([128, T], fp32, tag="psY")
            for idx, st in enumerate(ORDER):
                if interleave is not None and idx in interleave:
                    interleave[idx]()
                s_ps = psA.tile([128, T], fp32, tag="psA")
                for lc in range(2):
                    nc.tensor.matmul(
                        s_ps[:, lc * 512:(lc + 1) * 512],
                        k_lhsT(h, st),
                        qT[p0:p0 + D, h // 2, lc * 512:(lc + 1) * 512],
                        start=True,
                        stop=True,
                    )
                at = attn_pool.tile([128, T], bf16, tag="attn")
                nc.scalar.activation(
                    at[:], s_ps[:], Exp, bias=wb[:, st:st + 1], scale=scale
                )
                for lc in range(2):
                    nc.tensor.matmul(
                        y_ps[0:VW, lc * 512:(lc + 1) * 512],
                        v_lhsT(h, st),
                        at[:, lc * 512:(lc + 1) * 512],
                        start=(idx == 0),
                        stop=(idx == ST - 1),
                    )
            # evacuate y' to SBUF promptly (frees the single psY slot), then
            # normalize: yT[d, l] = y'[d, l] * (1 / y'[64, l]).  The last head
            # skips the evacuation copy (nobody waits on its psY slot) and
            # normalizes in l-halves so the last output round starts earlier.
            if h == H - 1:
                for lc in range(2):
                    ls = slice(lc * 512, (lc + 1) * 512)
                    recip = misc.tile([128, T], fp32, tag="recip")
                    nc.vector.reciprocal(recip[0:1, ls], y_ps[D:D + 1, ls])
                    rb = misc.tile([128, T], fp32, tag="rb")
                    nc.gpsimd.partition_broadcast(rb[0:D, ls], recip[0:1, ls])
                    nc.vector.tensor_mul(
                        yT[p0:p0 + D, h // 2, ls], y_ps[0:D, ls], rb[0:D, ls]
                    )
                    if tail_cb is not None:
                        tail_cb(lc)
                return
            ysrc = misc.tile([128, T], fp32, tag="ycp")
            nc.vector.tensor_copy(ysrc[0:VW, :], y_ps[0:VW, :])
            recip = misc.tile([128, T], fp32, tag="recip")
            nc.vector.reciprocal(recip[0:1, :], ysrc[D:D + 1, :])
            rb = misc.tile([128, T], fp32, tag="rb")
            nc.gpsimd.partition_broadcast(rb[0:D, :], recip[0:1, :])
            nc.vector.tensor_mul(yT[p0:p0 + D, h // 2, :], ysrc[0:D, :], rb[0:D, :])

        # incremental output projection: round cc computes the partial
        # out += yT[c-chunk cc] @ wp[cc] once heads 2cc and 2cc+1 are done.
        def out_round(cc, tts=None):
            for tt in (range(T // 128) if tts is None else tts):
                ps = psP.tile([128, 512], fp32, tag="psP")
                nc.tensor.matmul(
                    ps[:],
                    yT[:, cc, tt * 128:(tt + 1) * 128],
                    wp[:, cc, :],
                    start=True,
                    stop=True,
                )
                if cc == 0:
                    nc.vector.tensor_copy(out_acc[:, tt, :], ps[:])
                else:
                    nc.vector.tensor_add(out_acc[:, tt, :], out_acc[:, tt, :], ps[:])
                if cc == CT - 1:
                    nc.sync.dma_start(out_d[tt * 128:(tt + 1) * 128, :],
                                      out_acc[:, tt, :])

        # ---------------- emission order (overlap projections under head 0) --
        # Head 0 walks own-block tiles first (ORDER), and the remaining slot
        # projections are emitted just-in-time inside its loop — each write
        # strictly precedes its first read in program order (Tile has
        # sequential semantics: a read emitted before the write would see
        # uninitialized SBUF and force the write to wait via WAR).
        # Minimal pre-attention lead-in: head 0's own-block tiles need only
        # kT jt=0 and vown.  Slot projections stream in just-in-time inside
        # head 0 (each emitted 4 s-tiles before its first read); the other kT
        # jt tiles are emitted before the head pair that reads them.
        proj_kT_own([0])
        proj_v_own()
        attn_head(0, interleave={
            4: lambda: proj_slot(0),
            8: lambda: proj_slot(1),
            12: lambda: proj_slot(2),
            16: lambda: proj_slot(3),
            20: lambda: proj_slot(4),
            24: lambda: proj_slot(5),
        })
        proj_kT_own([1])
        attn_head(1)
        out_round(0)
        attn_head(2, interleave={0: lambda: proj_kT_own([2])})
        attn_head(3)
        out_round(1)
        attn_head(4, interleave={0: lambda: proj_kT_own([3])})
        attn_head(5)
        out_round(2)
        attn_head(6)
        attn_head(7, tail_cb=lambda lc: out_round(3, range(lc * 4, lc * 4 + 4)))


def _build_bass():
    import concourse.tile as tile
    from concourse import bacc, mybir

    nc = bacc.Bacc("TRN2", debug=False, target_bir_lowering=False)
    with tile.TileContext(nc) as tc:
        _emit(nc, tc, mybir)
    nc.compile()
    return nc


def _slots_and_bias(b):
    """Memory slots (6) + weights, and the tail weight, for batch b."""
    mem = [((b * 7 + m) % 8 - (b * 7 + m) // 8) % 8 for m in range(7)]
    tail_w = 1 + sum(1 for s in mem if s == b)
    counts = {}
    order = []
    for s in mem:
        if s == b:
            continue
        if s not in counts:
            counts[s] = 0
            order.append(s)
        counts[s] += 1
    slots = [(s, counts[s]) for s in order]
    assert len(slots) <= NSLOT, (b, slots)
    while len(slots) < NSLOT:
        slots.append((b, 0))  # padding slot: weight 0 (bias -60 -> exp ~ 0)
    bias = np.zeros(ST, np.float32)
    for m, (_, w) in enumerate(slots):
        bias[m * 4:(m + 1) * 4] = np.log(w) if w > 0 else -60.0
    bias[NSLOT * 4:NSLOT * 4 + 4] = np.log(tail_w)  # own first half
    # own second half (last 4 tiles) keeps bias 0 (weight 1)
    return slots, bias


def _prep_inputs(x, q, W_kv, W_proj):
    def bf(a):
        return np.ascontiguousarray(a.astype(BF16))

    def f8(a):
        return np.ascontiguousarray(a.astype(FP8))

    hcast = f8 if FP8_SLOTS else bf
    wk = bf(W_kv[:, :C])
    wv = bf(W_kv[:, C:])
    wp = bf(W_proj)
    in_maps = []
    for b in range(NCORES):
        slots, bias = _slots_and_bias(b)
        m = {
            "xT": bf(x[b].T),
            "qT": bf(q[b].T),
            "xhT": np.stack([hcast(x[s, :T2, :].T) for s, _ in slots]),
            "wbias": np.ascontiguousarray(
                np.broadcast_to(bias, (128, ST)).astype(np.float32)
            ),
            "wk": wk, "wv": wv, "wp": wp,
        }
        if FP8_SLOTS:
            m["wk8"] = f8(W_kv[:, :C])
            m["wv8"] = f8(W_kv[:, C:])
        in_maps.append(m)
    return in_maps


def kernel(x, q, W_kv, W_proj):
    global LAST_RESULTS
    from concourse.bass_utils import run_bass_kernel_spmd

    if "nc" not in _CACHE:
        _CACHE["nc"] = _build_bass()
    nc = _CACHE["nc"]

    x = np.asarray(x, dtype=np.float32)
    q = np.asarray(q, dtype=np.float32)
    W_kv = np.asarray(W_kv, dtype=np.float32)
    W_proj = np.asarray(W_proj, dtype=np.float32)

    in_maps = _prep_inputs(x, q, W_kv, W_proj)
    trace = bool(int(os.environ.get("KERNEL_TRACE", "0")))
    res = run_bass_kernel_spmd(nc, in_maps, core_ids=list(range(NCORES)), trace=trace)
    LAST_RESULTS = res
    out = np.stack([np.asarray(res.results[b]["out"], dtype=np.float32)
                    for b in range(NCORES)])
    return out
